# revision 11
# baseline (speedup 1.0000x reference)
"""Multi-head causal self-attention (S=4096, D=2048, H=16) on 8 trn2 NeuronCores.

Sharding: tensor-parallel over heads — 2 heads per core. Each core computes
q/k/v projections for its head group, causal flash-style attention, and its
partial out-projection; the host sums the 8 partials (the "all-reduce").

Layout strategy (per core, all matmul operands bf16, PSUM accumulate fp32):
  - host pre-transposes x -> xT [D, S] (bf16) so the contraction dim d lands
    on SBUF partitions with no on-device transposes.
  - projections produce qT, kT in [hd, s] layout and v in natural [s, hd]
    layout (v via xT tiles as the stationary operand).
  - scores are computed transposed: sT[sk, sq] = k @ qT, so the PV matmul
    (lhsT = v natural tile) needs no transpose either.
  - softmax: no max subtraction (scores are O(1) for this problem), exp on
    ScalarE with the 1/sqrt(hd) scale folded in, denominator via an M=1
    ones-matmul accumulated in PSUM (2 chunk-head slots share one bank via
    the partition dim), normalization after PV via reciprocal_approx_fast +
    gpsimd partition-broadcast + one vector multiply.
  - biases: bq/bk added per-partition on the [hd, s] tiles; bv folds into a
    host-side rank-1 correction (softmax rows sum to 1); bo added on host.
"""

import numpy as np

S, D, H = 4096, 2048, 16
HD = D // H  # 128
N_CORES = 8
HPC = H // N_CORES  # heads per core = 2
DPC = HPC * HD  # head dims per core = 256
SCALE = 1.0 / np.sqrt(np.float32(HD))

SQ = 512  # sq chunk width
NSQ = S // SQ  # 8
NKT = S // 128  # 32 sk tiles
NDT = D // 128  # 16 d tiles

_CACHE = {}


def _build(reps: int = 1):
    import concourse.bacc as bacc
    import concourse.mybir as mybir
    import concourse.tile as tile

    f32 = mybir.dt.float32
    bf16 = mybir.dt.bfloat16

    nc = bacc.Bacc("TRN2", target_bir_lowering=False)

    xT = nc.dram_tensor("xT", [D, S], bf16, kind="ExternalInput")
    wq = nc.dram_tensor("wq", [D, DPC], bf16, kind="ExternalInput")
    wk = nc.dram_tensor("wk", [D, DPC], bf16, kind="ExternalInput")
    wv = nc.dram_tensor("wv", [D, DPC], bf16, kind="ExternalInput")
    wo = nc.dram_tensor("wo", [DPC, D], bf16, kind="ExternalInput")
    bqk = nc.dram_tensor("bqk", [2, DPC], f32, kind="ExternalInput")
    masks = nc.dram_tensor("masks", [128, 2048], bf16, kind="ExternalInput")
    ones_in = nc.dram_tensor("ones", [128, 1], bf16, kind="ExternalInput")
    out = nc.dram_tensor("out", [S, D], f32, kind="ExternalOutput")

    xT3 = xT.rearrange("(dt p) s -> p dt s", p=128)
    out3 = out.rearrange("(st p) d -> p st d", p=128)

    with tile.TileContext(nc) as tc:
        with (
            tc.tile_pool(name="persist", bufs=1) as persist,
            tc.tile_pool(name="misc", bufs=1) as misc,
        ):
            # persistent SBUF tensors
            qT = persist.tile([128, HPC, S], bf16, tag="qT")
            kT = persist.tile([128, HPC, S], bf16, tag="kT")
            vn = persist.tile([128, NKT, DPC], bf16, tag="vn")  # v natural [sk, hd]
            wo_sb = persist.tile([128, HPC, D], bf16, tag="wo")
            mask_sb = persist.tile([128, 4, SQ], bf16, tag="mask")
            bias_sb = misc.tile([128, 2, HPC], f32, tag="bias")  # [.,0,.]=bq [.,1,.]=bk
            ones_sb = misc.tile([128, 1], bf16, tag="ones")

            if reps == 1:
                _phases(nc, tc, mybir, f32, bf16, qT, kT, vn, wo_sb, mask_sb,
                        bias_sb, ones_sb, xT3, out3, wq, wk, wv, wo, masks,
                        bqk, ones_in)
            else:
                with tc.For_i(0, reps, 1):
                    _phases(nc, tc, mybir, f32, bf16, qT, kT, vn, wo_sb,
                            mask_sb, bias_sb, ones_sb, xT3, out3, wq, wk, wv,
                            wo, masks, bqk, ones_in)
    nc.finalize()
    return nc


def _phases(nc, tc, mybir, f32, bf16, qT, kT, vn, wo_sb, mask_sb, bias_sb,
            ones_sb, xT3, out3, wq, wk, wv, wo, masks, bqk, ones_in):
    # ---------------- Phase 1: projections ----------------
    with (
        tc.tile_pool(name="wproj", bufs=1) as wproj,
        tc.tile_pool(name="xin", bufs=2) as xin,
        tc.tile_pool(name="psp", bufs=1, space="PSUM") as psp,
    ):
        wq_sb = wproj.tile([128, NDT, DPC], bf16, tag="wq")
        wk_sb = wproj.tile([128, NDT, DPC], bf16, tag="wk")
        wv_sb = wproj.tile([128, NDT, DPC], bf16, tag="wv")
        # DMA order = priority under bandwidth saturation: the first matmuls
        # need the leading quarters of wq/wk/wv plus the first xt quarter;
        # wo/masks are only needed from phase 2 on, so their DMAs are emitted
        # inside the chunk loop (executed while phase 1 computes).
        wq4 = wq.rearrange("(dt p) m -> p dt m", p=128)
        wk4 = wk.rearrange("(dt p) m -> p dt m", p=128)
        wv4 = wv.rearrange("(dt p) m -> p dt m", p=128)
        # chunk-0 first-half x tile goes first: it gates the very first matmul
        xt_first = xin.tile([128, NDT // 2, SQ], bf16, tag="xt", name="xt_first")
        nc.sync.dma_start(out=wq_sb[:, 0:4, :], in_=wq4[:, 0:4, :])
        for q in range(2):
            nc.sync.dma_start(
                out=xt_first[:, 4 * q : 4 * q + 4, :],
                in_=xT3[:, 4 * q : 4 * q + 4, 0:SQ],
            )
        for q in range(4):
            sl = slice(4 * q, 4 * q + 4)
            if q > 0:
                nc.sync.dma_start(out=wq_sb[:, sl, :], in_=wq4[:, sl, :])
            nc.sync.dma_start(out=wk_sb[:, sl, :], in_=wk4[:, sl, :])
            nc.sync.dma_start(out=wv_sb[:, sl, :], in_=wv4[:, sl, :])
        nc.sync.dma_start(
            out=bias_sb[:], in_=bqk.rearrange("b (h p) -> p b h", p=128)
        )
        nc.sync.dma_start(out=ones_sb[:], in_=ones_in[:])

        for j in range(NSQ):
            if j == 1:  # phase-2-only inputs: load during phase-1 compute
                nc.sync.dma_start(
                    out=mask_sb[:], in_=masks.rearrange("p (m n) -> p m n", m=4)
                )
                nc.sync.dma_start(
                    out=wo_sb[:], in_=wo.rearrange("(h p) d -> p h d", p=128)
                )
            ps_q = [psp.tile([128, SQ], f32, tag=f"psq{h}", name=f"psq{h}") for h in range(HPC)]
            ps_k = [psp.tile([128, SQ], f32, tag=f"psk{h}", name=f"psk{h}") for h in range(HPC)]
            ps_v = [psp.tile([128, DPC], f32, tag=f"psv{i}", name=f"psv{i}") for i in range(4)]
            for half in range(2):
                if j == 0 and half == 0:
                    xt = xt_first
                else:
                    xt = xin.tile([128, NDT // 2, SQ], bf16, tag="xt")
                    for q in range(2):
                        nc.sync.dma_start(
                            out=xt[:, 4 * q : 4 * q + 4, :],
                            in_=xT3[:, half * 8 + 4 * q : half * 8 + 4 * q + 4,
                                    j * SQ : (j + 1) * SQ],
                        )
                for dtl in range(NDT // 2):
                    dt = half * 8 + dtl
                    st = dict(start=(dt == 0), stop=(dt == NDT - 1))
                    for h in range(HPC):
                        nc.tensor.matmul(
                            ps_q[h][:],
                            wq_sb[:, dt, h * 128 : h * 128 + 128],
                            xt[:, dtl, :],
                            **st,
                        )
                        nc.tensor.matmul(
                            ps_k[h][:],
                            wk_sb[:, dt, h * 128 : h * 128 + 128],
                            xt[:, dtl, :],
                            **st,
                        )
                    for i in range(4):
                        nc.tensor.matmul(
                            ps_v[i][:],
                            xt[:, dtl, i * 128 : i * 128 + 128],
                            wv_sb[:, dt, :],
                            **st,
                        )
            for h in range(HPC):
                nc.vector.tensor_scalar_add(
                    qT[:, h, j * SQ : (j + 1) * SQ],
                    ps_q[h][:],
                    bias_sb[:, 0, h : h + 1],
                )
                nc.vector.tensor_scalar_add(
                    kT[:, h, j * SQ : (j + 1) * SQ],
                    ps_k[h][:],
                    bias_sb[:, 1, h : h + 1],
                )
            for i in range(4):
                nc.vector.tensor_copy(vn[:, 4 * j + i, :], ps_v[i][:])

    # ---------------- Phase 2: attention + out-proj ----------------
    with (
        tc.tile_pool(name="expp", bufs=6) as expp,
        tc.tile_pool(name="otp", bufs=2) as otp,
        tc.tile_pool(name="outp", bufs=2) as outp,
        tc.tile_pool(name="rdp", bufs=2) as rdp,
        tc.tile_pool(name="pss", bufs=3, space="PSUM") as pss,
        tc.tile_pool(name="pso", bufs=2, space="PSUM") as pso,
        tc.tile_pool(name="psd", bufs=1, space="PSUM") as psd,
        tc.tile_pool(name="psb", bufs=2, space="PSUM") as psb,
    ):
        ps_d2 = psd.tile([1, SQ], f32, tag="psd")
        # Software-pipelined emitter. For each tile the TensorE order is
        # scores(t) then pv/denom(t-1), so the exp(t-1) latency on ScalarE is
        # hidden behind scores(t). Out-projection of chunk j-1 is drip-fed
        # into chunk j's tile stream (one sub-step per tile) so the h1
        # normalize tail never stalls TensorE.
        pending = []

        def make_outproj(j, oT):
            steps = []
            box = {}

            def step(n, si):
                def f():
                    if si == 0:
                        box[n] = outp.tile([128, 4, SQ], f32, tag="ob", name="ob")
                    ps_p = psb.tile([128, SQ], f32, tag="ps_r", name="ps_p")
                    for h2 in range(HPC):
                        nc.tensor.matmul(
                            ps_p[:],
                            oT[:, h2, si * 128 : si * 128 + 128],
                            wo_sb[:, h2, n * SQ : n * SQ + SQ],
                            start=(h2 == 0),
                            stop=(h2 == HPC - 1),
                        )
                    if si % 2 == 0:
                        nc.vector.tensor_copy(box[n][:, si, :], ps_p[:])
                    else:
                        nc.scalar.copy(box[n][:, si, :], ps_p[:])
                    if si == 3:
                        nc.sync.dma_start(
                            out=out3[:, 4 * j : 4 * j + 4, n * SQ : n * SQ + SQ],
                            in_=box[n][:],
                        )
                return f

            for n in range(4):
                for si in range(4):
                    steps.append(step(n, si))
            return steps

        # Descending chunk order: the phase transition primes the pipeline on
        # the longest tile stream, so per-head tails hide under real work;
        # the short chunks run last where a drain is unavoidable anyway.
        for j in range(NSQ - 1, -1, -1):
            sq = slice(j * SQ, (j + 1) * SQ)
            ntiles = 4 * (j + 1)
            oT = otp.tile([128, HPC, SQ], bf16, tag="oT")
            stream = [(h, t) for h in range(HPC) for t in range(ntiles)]
            ps_os = {}
            exs = {}
            dpend = [None]

            def lo_of(t):
                # diagonal tiles keep only columns >= delta; skip the dead ones
                return max(0, (t - (ntiles - 4)) * 128)

            def drain(idx):
                ph, pt = stream[idx]
                ps_o = ps_os[ph]
                lo = lo_of(pt)
                nc.tensor.matmul(
                    ps_o[:, lo:SQ],
                    vn[:, pt, ph * 128 : ph * 128 + 128],
                    exs[(ph, pt)][:, lo:SQ],
                    start=(pt == 0),
                    stop=(pt == ntiles - 1),
                )
                if pt % 2 == 1:
                    # bf16 pair-sum on DVE halves the ones-matmul count; the
                    # ones-matmul itself lags one more tile so the DVE add is
                    # off TensorE's critical path.
                    lo0, lo1 = lo_of(pt - 1), lo_of(pt)
                    exp2 = expp.tile([128, SQ], bf16, tag="ex2", bufs=3, name="exp2")
                    nc.vector.tensor_add(
                        exp2[:, lo1:SQ],
                        exs[(ph, pt - 1)][:, lo1:SQ],
                        exs[(ph, pt)][:, lo1:SQ],
                    )
                    if lo1 > lo0:
                        nc.vector.tensor_copy(
                            exp2[:, lo0:lo1], exs[(ph, pt - 1)][:, lo0:lo1]
                        )
                    if dpend[0] is not None:
                        nc.tensor.matmul(*dpend[0][0], **dpend[0][1])
                    dpend[0] = (
                        (ps_d2[:, lo0:SQ], ones_sb[:], exp2[:, lo0:SQ]),
                        dict(start=(pt == 1), stop=(pt == ntiles - 1)),
                    )
                if pt == ntiles - 1 and dpend[0] is not None:
                    nc.tensor.matmul(*dpend[0][0], **dpend[0][1])
                    dpend[0] = None
                if pt == ntiles - 1:  # head complete: normalize tail
                    rd = rdp.tile([1, SQ], f32, tag="rd", name="rd")
                    nc.vector.reciprocal_approx_fast(rd[:], ps_d2[:])
                    rdb = rdp.tile([128, SQ], f32, tag="rdb", name="rdb")
                    nc.gpsimd.partition_broadcast(rdb[:], rd[:])
                    nc.vector.tensor_mul(oT[:, ph, :], ps_o[:], rdb[:])

            for i, (h, t) in enumerate(stream):
                if t == 0:
                    ps_os[h] = pso.tile([128, SQ], f32, tag="ps_o", name="ps_o")
                lo = lo_of(t)
                ps_s = pss.tile([128, SQ], f32, tag="ps_s", name="ps_s")
                nc.tensor.matmul(
                    ps_s[:, lo:SQ],
                    kT[:, h, t * 128 : t * 128 + 128],
                    qT[:, h, j * SQ + lo : (j + 1) * SQ],
                    start=True,
                    stop=True,
                )
                ex = expp.tile([128, SQ], bf16, tag="ex", name="ex")
                if t >= ntiles - 4:  # diagonal block: apply causal mask
                    ex0 = expp.tile([128, SQ], bf16, tag="exm", bufs=2, name="ex0")
                    nc.scalar.activation(
                        ex0[:, lo:SQ], ps_s[:, lo:SQ],
                        mybir.ActivationFunctionType.Exp, scale=float(SCALE),
                    )
                    nc.vector.tensor_mul(
                        ex[:, lo:SQ], ex0[:, lo:SQ],
                        mask_sb[:, t - (ntiles - 4), lo:SQ],
                    )
                else:
                    nc.scalar.activation(
                        ex[:], ps_s[:], mybir.ActivationFunctionType.Exp,
                        scale=float(SCALE),
                    )
                exs[(h, t)] = ex
                if i > 0:
                    drain(i - 1)
                if pending:
                    pending.pop(0)()
            drain(len(stream) - 1)
            for f in pending:  # leftovers (next stream shorter than 16)
                f()
            pending = make_outproj(j, oT)
        for f in pending:
            f()


def _get_nc():
    if "nc" not in _CACHE:
        _CACHE["nc"] = _build()
    return _CACHE["nc"]


def _host_masks() -> np.ndarray:
    # mask block m covers the diagonal sk tile with delta = 128*m:
    # keep iff (c >= p + delta), delta in {0,128,256,384}.
    p = np.arange(128)[:, None]
    c = np.arange(SQ)[None, :]
    blocks = []
    for delta in (0, 128, 256, 384):
        blocks.append((c >= p + delta).astype(np.float32))
    m = np.concatenate(blocks, axis=1)  # [128, 2048]
    return np.ascontiguousarray(m)


def make_in_maps(inputs: dict) -> list:
    import concourse.mybir as mybir

    BF16 = mybir.dt.np(mybir.dt.bfloat16)
    Wq, bq = np.asarray(inputs["Wq"], np.float32), np.asarray(inputs["bq"], np.float32)
    Wk, bk = np.asarray(inputs["Wk"], np.float32), np.asarray(inputs["bk"], np.float32)
    Wv = np.asarray(inputs["Wv"], np.float32)
    Wo = np.asarray(inputs["Wo"], np.float32)
    xT = np.ascontiguousarray(
        np.asarray(inputs["hidden_states"], np.float32).T
    ).astype(BF16)
    masks = _host_masks().astype(BF16)
    ones = np.ones((128, 1), BF16)
    in_maps = []
    for c in range(N_CORES):
        r = slice(c * DPC, (c + 1) * DPC)
        in_maps.append(
            {
                "xT": xT,
                "wq": np.ascontiguousarray(Wq[r, :].T).astype(BF16),
                "wk": np.ascontiguousarray(Wk[r, :].T).astype(BF16),
                "wv": np.ascontiguousarray(Wv[r, :].T).astype(BF16),
                "wo": np.ascontiguousarray(Wo[:, r].T).astype(BF16),
                "bqk": np.stack([bq[r], bk[r]]),
                "masks": masks,
                "ones": ones,
            }
        )
    return in_maps


def kernel(hidden_states, Wq, bq, Wk, bk, Wv, bv, Wo, bo):
    from concourse.bass_utils import run_bass_kernel_spmd

    Wv, bv = np.asarray(Wv, np.float32), np.asarray(bv, np.float32)
    Wo, bo = np.asarray(Wo, np.float32), np.asarray(bo, np.float32)
    in_maps = make_in_maps(
        dict(hidden_states=hidden_states, Wq=Wq, bq=bq, Wk=Wk, bk=bk, Wv=Wv, Wo=Wo)
    )

    nc = _get_nc()
    results = run_bass_kernel_spmd(nc, in_maps, core_ids=list(range(N_CORES))).results

    acc = results[0]["out"].astype(np.float32)
    for c in range(1, N_CORES):
        acc += results[c]["out"]
    # bias corrections: bo plus the deferred bv contribution (attn rows sum to 1)
    acc += (bo + bv @ Wo.T)[None, :]
    return acc
